# revision 39
# baseline (speedup 1.0000x reference)
"""Bass/Trainium2 kernel for nn_MixedAttentionBlock_80290118632213.

Swin-style block: LN -> shifted-window attention (+rel-pos bias) -> residual
-> channel-attention branch (SE + depthwise 3x3) -> residual -> MLP (erf GELU)
-> residual.  Data-parallel over batch: 32 samples -> 8 cores x 4 samples.

The axon relay moves ~35-58 MB/s, so wall time is transport-bound; the wire
protocol keeps every big tensor at 1 byte/element:
  - x is int8-quantized on the host with one global scale (q = rint(x/s)),
    pre-rolled by (-4,-4); the device consumes q directly (LN1 is scale-
    invariant given eps/s^2; residual sites multiply by s)
  - the device returns delta = out - x as per-token uint8 (u = rne(
    -delta*126.99/amax + 128)) plus the per-token amax; the host reconstructs
    out = x_f32 + (128-u)*amax/126.99 with the un-roll fused into quadrant
    adds, so the exact f32 x never loses precision on the wire
  - weights/consts are device-cached keyed by a content hash; donated output
    buffers are zero-filled on-device; result shards are fetched by an I/O
    thread pool while the (single-CPU) host dequantizes serially

Device layout strategy per sample:
  - token-major [128 tok, 256] tiles for LN / residuals (per-token stats are
    per-partition scalars)
  - bf16 DMA transposes to feature-major [256, 4096] for all GEMMs
    (out.T = W @ x.T keeps both operands and results feature-major)
  - attention runs in transposed-score space: S'[m,n] = K Q^T via
    tile_position-packed (4 heads x 2 windows) PE matmuls, softmax-over-
    partitions done with a blockdiag-ones matmul + reciprocal + a broadcast
    matmul, AV consumes the normalized weights directly (no transposes)
  - depthwise 3x3 via chained scalar_tensor_tensor in a zero-padded
    original-raster workspace
"""

import sys

import numpy as np

sys.path.insert(0, "/opt/trn_rl_repo")

import ml_dtypes  # noqa: E402

import concourse.bass as bass  # noqa: E402
import concourse.tile as tile  # noqa: E402
from concourse import bacc, mybir  # noqa: E402
from concourse.bass_utils import run_bass_kernel_spmd  # noqa: E402
from contextlib import ExitStack  # noqa: E402

F32 = mybir.dt.float32
F32R = mybir.dt.float32r
BF16 = mybir.dt.bfloat16
I8 = mybir.dt.int8
U8 = mybir.dt.uint8
AF = mybir.ActivationFunctionType
OP = mybir.AluOpType
AX = mybir.AxisListType

B, H, W, C = 32, 64, 64, 256
NH, WS, SS = 8, 8, 4
N = WS * WS                  # 64 tokens per window
NW = (H // WS) * (W // WS)   # 64 windows
HD = C // NH                 # 32
MLP_H = 4 * C                # 1024
CAB_H = C // 16              # 16
EPS = 1e-5
NCORES = 8
S = B // NCORES              # 4 samples per core
T = H * W                    # 4096
NT = T // 128                # 32 token tiles
NPAIR = NW // 2              # 32 window pairs
NCH = T // 512               # 8 chunks

# pair -> bias/mask table index (0 = bias only; >0 = bias + that pair's mask)
_MASKED_PAIRS = sorted({p for p in range(NPAIR) if p % 4 == 3 or p // 4 == 7})
_PAIR_TO_BM = {p: 0 for p in range(NPAIR)}
for _i, _p in enumerate(_MASKED_PAIRS):
    _PAIR_TO_BM[_p] = 1 + _i
NBM = 1 + len(_MASKED_PAIRS)

_prog_cache = {}


def _bf(x):
    return np.ascontiguousarray(np.asarray(x, np.float32).astype(ml_dtypes.bfloat16))


def _f32(x):
    return np.ascontiguousarray(np.asarray(x, np.float32))


def _rel_pos_index():
    coords = np.stack(np.meshgrid(np.arange(WS), np.arange(WS), indexing="ij"))
    flat = coords.reshape(2, -1)
    rel = (flat[:, :, None] - flat[:, None, :]).transpose(1, 2, 0)
    rel[:, :, 0] += WS - 1
    rel[:, :, 1] += WS - 1
    rel[:, :, 0] *= 2 * WS - 1
    return rel.sum(-1)


def _attn_mask():
    img = np.zeros((H, W))
    sl = (slice(0, -WS), slice(-WS, -SS), slice(-SS, None))
    cnt = 0
    for hs in sl:
        for ws in sl:
            img[hs, ws] = cnt
            cnt += 1
    mw = img.reshape(H // WS, WS, W // WS, WS).transpose(0, 2, 1, 3).reshape(-1, N)
    am = mw[:, None, :] - mw[:, :, None]
    return np.where(am != 0, -100.0, 0.0).astype(np.float32)


def _win_pair_view(fm_tile, pair):
    """[128, (w2, i, j)] gather of a window pair from a raster fm tile."""
    w0 = 2 * pair
    wi, wj0 = w0 // 8, w0 % 8
    v = fm_tile.rearrange("p (r c) -> p r c", r=64)
    v = v[:, wi * 8:(wi + 1) * 8, wj0 * 8:wj0 * 8 + 16]  # [p, i, (w2 j)]
    return v.rearrange("p i (w2 j) -> p w2 i j", w2=2)


def _win_one_view(fm_tile, w, r0, r1):
    """[r1-r0, (i, j)] gather of one window from fm tile rows r0:r1."""
    wi, wj = w // 8, w % 8
    v = fm_tile[r0:r1].rearrange("p (r c) -> p r c", r=64)
    return v[:, wi * 8:(wi + 1) * 8, wj * 8:(wj + 1) * 8]


def _emit(tc, dram, out_d, osc_d):
    nc = tc.nc
    x_d = dram["x"]

    with ExitStack() as ctx:
        con = ctx.enter_context(tc.tile_pool(name="con", bufs=1))
        big = ctx.enter_context(tc.tile_pool(name="big", bufs=1))
        st = ctx.enter_context(tc.tile_pool(name="st", bufs=3))
        sm = ctx.enter_context(tc.tile_pool(name="sm", bufs=2))
        smax = ctx.enter_context(tc.tile_pool(name="smax", bufs=2))
        h1p = ctx.enter_context(tc.tile_pool(name="h1p", bufs=1))
        cv = ctx.enter_context(tc.tile_pool(name="cv", bufs=2))
        ps = ctx.enter_context(tc.tile_pool(name="ps", bufs=8, space="PSUM"))

        def load_const(name):
            d = dram[name]
            t = con.tile(list(d.shape), d.dtype, tag=name, name=name)
            nc.sync.dma_start(out=t[:], in_=d.ap())
            return t

        wqk = load_const("wqk")
        qkb = load_const("qkb")
        wv = load_const("wv")
        vb = load_const("vb")
        wp = load_const("wp")
        pb = load_const("pb")
        f1 = load_const("f1")
        f1b = load_const("f1b")
        f2 = load_const("f2")
        f2b = load_const("f2b")
        blk = load_const("blk")
        blkt = load_const("blkt")
        cg = load_const("cg")
        cb_t = load_const("cb")
        cw1 = load_const("cw1")
        c1b = load_const("c1b")
        cw2 = load_const("cw2")
        c2b = load_const("c2b")
        dw = load_const("dw")
        dwb = load_const("dwb")
        xmeta = load_const("xmeta")  # [128,2]: col0 x dequant scale, col1 eps/s^2
        xsc_t = xmeta[:, 0:1]
        epsq_t = xmeta[:, 1:2]

        eps_t = con.tile([128, 1], F32)
        nc.vector.memset(eps_t, EPS)
        qoff_t = con.tile([128, 1], F32)
        nc.vector.memset(qoff_t, 128.0)

        # padded depthwise-conv workspace (shared across channel halves);
        # border stays zero forever
        ya_sp = con.tile([128, 66 * 66], BF16)
        nc.vector.memset(ya_sp[:], 0.0)
        ya3 = ya_sp.rearrange("p (r c) -> p r c", r=66)

        def ln_tile(xs, dst, gb=None, eps=None):
            """One-token-tile LayerNorm: xs [128,256] -> dst (bf16).
            LN is scale-invariant, so int8 q = x/s works directly when eps
            is rescaled to eps/s^2 (pass eps=epsq_t)."""
            stats = sm.tile([128, 6], F32, tag="lnstats")
            nc.vector.bn_stats(out=stats[:], in_=xs)
            mv = sm.tile([128, 2], F32, tag="lnmv")
            nc.vector.bn_aggr(out=mv[:], in_=stats[:])
            rs = sm.tile([128, 1], F32, tag="lnrs")
            nc.scalar.activation(out=rs[:], in_=mv[:, 1:2], func=AF.Sqrt,
                                 bias=(eps_t if eps is None else eps)[:],
                                 scale=1.0)
            nc.vector.reciprocal(out=rs[:], in_=rs[:])
            if gb is None:
                nm = sm.tile([128, 1], F32, tag="lnnm")
                nc.vector.tensor_scalar(
                    out=nm[:], in0=mv[:, 0:1], scalar1=rs[:], scalar2=-1.0,
                    op0=OP.mult, op1=OP.mult)
                nc.scalar.activation(out=dst, in_=xs, func=AF.Identity,
                                     bias=nm[:], scale=rs[:])
            else:
                z = sm.tile([128, C], F32, tag="lnz")
                nc.vector.tensor_scalar(
                    out=z[:], in0=xs, scalar1=mv[:, 0:1], scalar2=rs[:],
                    op0=OP.subtract, op1=OP.mult)
                zg = sm.tile([128, C], F32, tag="lnzg")
                nc.gpsimd.tensor_tensor(out=zg[:], in0=z[:], in1=gb[0][:],
                                        op=OP.mult)
                nc.gpsimd.tensor_tensor(out=dst, in0=zg[:], in1=gb[1][:],
                                        op=OP.add)

        def ln_to_fm(src_view, fm_tiles, gb=None, eps=None):
            """LN each token tile of src_view [128, NT, 256] and transpose the
            bf16 result into feature-major fm_tiles (2 x [128, T])."""
            for k in range(NT):
                lt = st.tile([128, C], BF16, tag="lntile")
                ln_tile(src_view[:, k, :], lt[:], gb=gb, eps=eps)
                for cbi in range(2):
                    nc.sync.dma_start_transpose(
                        fm_tiles[cbi][:, k * 128:(k + 1) * 128],
                        lt[:, cbi * 128:(cbi + 1) * 128])

        def tok_tile(fm_tiles, k, tag="tokT"):
            """Feature-major chunk k -> token-major [128, 256] bf16 tile."""
            dst = st.tile([128, C], BF16, tag=tag)
            for cbi in range(2):
                nc.sync.dma_start_transpose(
                    dst[:, cbi * 128:(cbi + 1) * 128],
                    fm_tiles[cbi][:, k * 128:(k + 1) * 128])
            return dst

        for s in range(S):
            # ---- load x (rolled raster, token-major, int8 q = x/s) --------
            x8 = big.tile([128, NT, C], I8, tag="x0")
            nc.sync.dma_start(
                out=x8[:],
                in_=x_d.ap()[s].rearrange("(k p) c -> p k c", p=128))
            # f32 accumulator for x after residuals
            x_sb = big.tile([128, NT, C], F32, tag="x_sb")

            # ---- LN1 -> fm (raster), then raster -> window-order shuffle --
            # LN directly on q (scale-invariant; eps rescaled)
            xr = [big.tile([128, T], BF16, tag=f"qkt{i}", name=f"xr{i}") for i in range(2)]
            ln_to_fm(x8, xr, eps=epsq_t)
            xt = [big.tile([128, T], BF16, tag=f"xt{i}", name=f"xt{i}") for i in range(2)]
            for cbi in range(2):
                srcv = xr[cbi].rearrange("p (wi i wj j) -> p wi wj i j",
                                         wi=8, i=8, wj=8)
                dstv = xt[cbi].rearrange("p (wi wj i j) -> p wi wj i j",
                                         wi=8, wj=8, i=8)
                nc.vector.tensor_copy(out=dstv, in_=srcv)

            # ---- QK^T feature-major [512, T] ------------------------------
            qkt = [big.tile([128, T], BF16, tag=f"qkt{i}", name=f"qkt{i}") for i in range(4)]
            for mt in range(4):
                for ch in range(NCH):
                    pt = ps.tile([128, 512], F32, tag="bank")
                    for k in range(2):
                        nc.tensor.matmul(
                            pt[:], wqk[:, k, mt * 128:(mt + 1) * 128],
                            xt[k][:, ch * 512:(ch + 1) * 512],
                            start=(k == 0), stop=(k == 1))
                    nc.scalar.activation(
                        out=qkt[mt][:, ch * 512:(ch + 1) * 512], in_=pt[:],
                        func=AF.Identity, bias=qkb[:, mt:mt + 1], scale=1.0)

            # ---- V token-major window-gathered [128, pair, 256] -----------
            v_sb = big.tile([128, NPAIR, C], BF16, tag="v_sb")
            for pair in range(NPAIR):
                pt = ps.tile([128, 512], F32, tag="bank")
                pv = pt[:, :C]
                for k in range(2):
                    nc.tensor.matmul(
                        pv, xt[k][:, pair * 128:(pair + 1) * 128], wv[:, k, :],
                        start=(k == 0), stop=(k == 1))
                nc.vector.scalar_tensor_tensor(
                    out=v_sb[:, pair, :], in0=pv, scalar=1.0, in1=vb[:],
                    op0=OP.mult, op1=OP.add)

            # ---- attention + proj, one group (4 pairs = 512 cols) at a time
            ptr_ = [big.tile([128, T], BF16, tag=f"xt{i}", name=f"ptr{i}") for i in range(2)]
            for g in range(NCH):
                otg = [st.tile([128, 512], BF16, tag=f"otg{i}", name=f"otg{i}") for i in range(2)]
                for pair in range(4 * g, 4 * g + 4):
                    col = (pair % 4) * 128
                    # KQ^T: 4 separate banks, one per row group (h4);
                    # columns of the softmax buffers are (h4, hw, n) ordered
                    sbank = [ps.tile([128, 128], F32, tag="bank",
                                     name=f"sb{h4}") for h4 in range(4)]
                    for hw in range(2):
                        for h4 in range(4):
                            h = hw * 4 + h4
                            for w2 in range(2):
                                w = 2 * pair + w2
                                lhsT = qkt[2 + hw][h4 * 32:h4 * 32 + 32,
                                                   w * 64:(w + 1) * 64]
                                rhs = qkt[hw][h4 * 32:h4 * 32 + 32,
                                              w * 64:(w + 1) * 64]
                                nc.tensor.matmul(
                                    sbank[h4][w2 * 64:(w2 + 1) * 64,
                                              hw * 64:(hw + 1) * 64],
                                    lhsT, rhs, start=True, stop=True,
                                    tile_position=(h4 * 32, w2 * 64))
                    bmt = st.tile([128, 512], BF16, tag="bmt", name="bmt")
                    nc.sync.dma_start(
                        out=bmt[:],
                        in_=dram["bmsk"].ap()[:, _PAIR_TO_BM[pair], :])
                    ss = smax.tile([128, 512], F32, tag="ss")
                    for h4 in range(4):
                        nc.vector.scalar_tensor_tensor(
                            out=ss[:, h4 * 128:(h4 + 1) * 128],
                            in0=sbank[h4][:], scalar=1.0,
                            in1=bmt[:, h4 * 128:(h4 + 1) * 128],
                            op0=OP.mult, op1=OP.add)
                    ex = smax.tile([128, 512], BF16, tag="ex")
                    nc.scalar.activation(out=ex[:], in_=ss[:], func=AF.Exp)
                    ps_r = ps.tile([2, 512], F32, tag="bank")
                    nc.tensor.matmul(ps_r[:], blk[:], ex[:],
                                     start=True, stop=True)
                    rr = smax.tile([2, 512], F32R, tag="rr")
                    with nc.allow_low_precision(reason="f32r storage is full fp32 bits"):
                        nc.vector.reciprocal(out=rr[:], in_=ps_r[:])
                    ps_rb = ps.tile([128, 512], F32, tag="bank")
                    nc.tensor.matmul(ps_rb[:], blkt[:], rr[:],
                                     start=True, stop=True)
                    en = smax.tile([128, 512], BF16, tag="en")
                    nc.vector.tensor_tensor(out=en[:], in0=ex[:],
                                            in1=ps_rb[:], op=OP.mult)
                    # AV: 2 banks, one per window row group (w2)
                    abank = [ps.tile([128, 128], F32, tag="bank",
                                     name=f"ab{w2}") for w2 in range(2)]
                    for hw in range(2):
                        for h4 in range(4):
                            h = hw * 4 + h4
                            for w2 in range(2):
                                lhsT = v_sb[w2 * 64:(w2 + 1) * 64, pair,
                                            h * 32:(h + 1) * 32]
                                rhs = en[w2 * 64:(w2 + 1) * 64,
                                         h4 * 128 + hw * 64:
                                         h4 * 128 + (hw + 1) * 64]
                                nc.tensor.matmul(
                                    abank[w2][h4 * 32:(h4 + 1) * 32,
                                              hw * 64:(hw + 1) * 64],
                                    lhsT, rhs, start=True, stop=True,
                                    tile_position=(w2 * 64, h4 * 32))
                    for hw in range(2):
                        for w2 in range(2):
                            nc.scalar.copy(
                                out=otg[hw][:, col + w2 * 64:
                                            col + (w2 + 1) * 64],
                                in_=abank[w2][:, hw * 64:(hw + 1) * 64])

                # ---- proj + window->raster shuffle for this group ---------
                ch = g
                for mt in range(2):
                    pt = ps.tile([128, 512], F32, tag="bank")
                    for k in range(2):
                        nc.tensor.matmul(
                            pt[:], wp[:, k, mt * 128:(mt + 1) * 128],
                            otg[k][:],
                            start=(k == 0), stop=(k == 1))
                    pm = st.tile([128, 512], BF16, tag=f"ptm{mt}")
                    if mt == 0:
                        nc.scalar.activation(
                            out=pm[:], in_=pt[:], func=AF.Identity,
                            bias=pb[:, mt:mt + 1], scale=1.0)
                    else:
                        nc.vector.tensor_scalar(
                            out=pm[:], in0=pt[:], scalar1=pb[:, mt:mt + 1],
                            scalar2=None, op0=OP.add)
                    # local wt (wj, i, j) -> raster (i, wj, j) within wi band
                    srcv = pm.rearrange("p (wj i j) -> p i wj j", wj=8, i=8)
                    dstv = ptr_[mt].rearrange("p (r c) -> p r c", r=64)
                    dstv = dstv[:, ch * 8:(ch + 1) * 8, :]
                    dstv = dstv.rearrange("p i (wj j) -> p i wj j", wj=8)
                    nc.gpsimd.tensor_copy(out=dstv, in_=srcv)

            # x1 = s*q + attn_out
            for k in range(NT):
                p_tok = tok_tile(ptr_, k)
                nc.vector.scalar_tensor_tensor(
                    out=x_sb[:, k, :], in0=x8[:, k, :], scalar=xsc_t[:],
                    in1=p_tok[:], op0=OP.mult, op1=OP.add)

            # ---- CAB ------------------------------------------------------
            y_fm = [big.tile([128, T], BF16, tag=f"xt{i}", name=f"yfm{i}") for i in range(2)]
            ln_to_fm(x_sb, y_fm, gb=(cg, cb_t))

            zstat = sm.tile([128, 2, 2], F32, tag="zstat")
            for cbi in range(2):
                nc.vector.tensor_reduce(
                    out=zstat[:, cbi, 0:1], in_=y_fm[cbi][:], axis=AX.X,
                    op=OP.add)
                nc.vector.tensor_reduce(
                    out=zstat[:, cbi, 1:2], in_=y_fm[cbi][:], axis=AX.X,
                    op=OP.max)
            # se(gap)+se(gmp): col0 = gap branch (sum x W1/T), col1 = gmp
            ps_se = ps.tile([CAB_H, 512], F32, tag="bank")
            for col in range(2):
                for cbi in range(2):
                    nc.tensor.matmul(
                        ps_se[:, col:col + 1],
                        cw1[:, cbi, col * CAB_H:(col + 1) * CAB_H],
                        zstat[:, cbi, col:col + 1],
                        start=(cbi == 0), stop=(cbi == 1))
            hse = sm.tile([CAB_H, 2], BF16, tag="hse")
            nc.scalar.activation(out=hse[:], in_=ps_se[:, 0:2], func=AF.Relu,
                                 bias=c1b[:], scale=1.0)
            ca = []
            for mt in range(2):
                ps_ca = ps.tile([128, 512], F32, tag="bank")
                nc.tensor.matmul(ps_ca[:, 0:2],
                                 cw2[:, mt * 128:(mt + 1) * 128],
                                 hse[:], start=True, stop=True)
                sig = sm.tile([128, 1], F32, tag=f"sig{mt}")
                nc.vector.tensor_reduce(out=sig[:], in_=ps_ca[:, 0:2],
                                        axis=AX.X, op=OP.add)
                ca_t = sm.tile([128, 1], F32, tag=f"ca{mt}")
                nc.scalar.activation(out=ca_t[:], in_=sig[:], func=AF.Sigmoid,
                                     bias=c2b[:, mt:mt + 1], scale=1.0)
                ca.append(ca_t)

            cab_fm = [big.tile([128, T], BF16, tag=f"qkt{i}", name=f"cabfm{i}") for i in range(2)]
            for cbi in range(2):
                ya4 = y_fm[cbi].rearrange("p (r c) -> p r c", r=64)
                # ya = y * ca fused into the rolled->original-raster scatter
                for (ro, rr_, nr) in ((4, 0, 60), (0, 60, 4)):
                    for (co, cc, ncol) in ((4, 0, 60), (0, 60, 4)):
                        nc.vector.tensor_scalar(
                            out=ya3[:, 1 + ro:1 + ro + nr,
                                    1 + co:1 + co + ncol],
                            in0=ya4[:, rr_:rr_ + nr, cc:cc + ncol],
                            scalar1=ca[cbi][:], scalar2=None, op0=OP.mult)
                yc = cv.tile([128, 64, 64], BF16, tag="yc", name="yc")
                eng = nc.vector
                first = True
                for di in range(3):
                    for dj in range(3):
                        tap = ya3[:, di:di + 64, dj:dj + 64]
                        prev = ya3[:, 1:65, 1:65] if first else yc[:]
                        eng.scalar_tensor_tensor(
                            out=yc[:], in0=tap,
                            scalar=dw[:, cbi, 3 * di + dj:3 * di + dj + 1],
                            in1=prev, op0=OP.mult, op1=OP.add)
                        first = False
                # original -> rolled raster, +dw bias, on ACT
                cr = cab_fm[cbi].rearrange("p (r c) -> p r c", r=64)
                for (ro, rr_, nr) in ((4, 0, 60), (0, 60, 4)):
                    for (co, cc, ncol) in ((4, 0, 60), (0, 60, 4)):
                        nc.scalar.activation(
                            out=cr[:, rr_:rr_ + nr, cc:cc + ncol],
                            in_=yc[:, ro:ro + nr, co:co + ncol],
                            func=AF.Identity, bias=dwb[:, cbi:cbi + 1],
                            scale=1.0)

            # x2 = x1 + cab_out
            for k in range(NT):
                c_tok = tok_tile(cab_fm, k)
                nc.gpsimd.tensor_tensor(
                    out=x_sb[:, k, :], in0=c_tok[:], in1=x_sb[:, k, :],
                    op=OP.add)

            # ---- MLP ------------------------------------------------------
            xt3 = [big.tile([128, T], BF16, tag=f"xt{i}", name=f"xt3_{i}") for i in range(2)]
            ln_to_fm(x_sb, xt3)

            for ch in range(NCH):
                h1 = [h1p.tile([128, 512], BF16, tag=f"h1_{mt}", name=f"h1_{mt}")
                      for mt in range(8)]
                for mt in range(8):
                    pt = ps.tile([128, 512], F32, tag="bank")
                    for k in range(2):
                        nc.tensor.matmul(
                            pt[:], f1[:, k, mt * 128:(mt + 1) * 128],
                            xt3[k][:, ch * 512:(ch + 1) * 512],
                            start=(k == 0), stop=(k == 1))
                    nc.scalar.activation(out=h1[mt][:], in_=pt[:],
                                         func=AF.Gelu,
                                         bias=f1b[:, mt:mt + 1], scale=1.0)
                h2c = [st.tile([128, 512], BF16, tag=f"h2c{mt}", name=f"h2c{mt}")
                       for mt in range(2)]
                for mt in range(2):
                    pt = ps.tile([128, 512], F32, tag="bank")
                    for k in range(8):
                        nc.tensor.matmul(
                            pt[:], f2[:, k, mt * 128:(mt + 1) * 128],
                            h1[k][:], start=(k == 0), stop=(k == 7))
                    if mt == 0:
                        nc.vector.tensor_scalar(
                            out=h2c[mt][:], in0=pt[:],
                            scalar1=f2b[:, mt:mt + 1], scalar2=None,
                            op0=OP.add)
                    else:
                        nc.scalar.activation(
                            out=h2c[mt][:], in_=pt[:], func=AF.Identity,
                            bias=f2b[:, mt:mt + 1], scale=1.0)
                # nd = x0 - (x2 + h2) = -delta, quantized per-token to uint8:
                # u = rne(nd*(126.99/amax) + 128); host: d = (128-u)*a/126.99
                for kk in range(4):
                    k = ch * 4 + kk
                    h_tok = tok_tile(h2c, kk, tag="htok")
                    o_t = st.tile([128, C], F32, tag="otile")
                    nc.vector.tensor_tensor(out=o_t[:], in0=x_sb[:, k, :],
                                            in1=h_tok[:], op=OP.add)
                    nd = st.tile([128, C], F32, tag="dtile")
                    nc.vector.scalar_tensor_tensor(
                        out=nd[:], in0=x8[:, k, :], scalar=xsc_t[:],
                        in1=o_t[:], op0=OP.mult, op1=OP.subtract)
                    qa = sm.tile([128, 1], F32, tag="qa")
                    nc.vector.tensor_reduce(out=qa[:], in_=nd[:], axis=AX.X,
                                            op=OP.max, apply_absolute_value=True)
                    qa2 = sm.tile([128, 1], F32, tag="qa2")
                    nc.vector.tensor_scalar(out=qa2[:], in0=qa[:],
                                            scalar1=1e-20, scalar2=None,
                                            op0=OP.max)
                    qr = sm.tile([128, 1], F32, tag="qr")
                    nc.vector.reciprocal(out=qr[:], in_=qa2[:])
                    qs = sm.tile([128, 1], F32, tag="qs")
                    nc.vector.tensor_scalar(out=qs[:], in0=qr[:],
                                            scalar1=126.99, scalar2=None,
                                            op0=OP.mult)
                    u_t = st.tile([128, C], U8, tag="utile")
                    nc.scalar.activation(out=u_t[:], in_=nd[:], func=AF.Identity,
                                         bias=qoff_t[:], scale=qs[:])
                    nc.sync.dma_start(
                        out=out_d.ap()[s, k * 128:(k + 1) * 128, :],
                        in_=u_t[:])
                    nc.sync.dma_start(
                        out=osc_d.ap()[s, k * 128:(k + 1) * 128, :],
                        in_=qa2[:])


def _build_program():
    nc = bacc.Bacc("TRN2", target_bir_lowering=False, debug=False)
    dram = {}

    def din(name, shape, dt=F32):
        dram[name] = nc.dram_tensor(name, list(shape), dt, kind="ExternalInput")
        return dram[name]

    din("x", (S, T, C), I8)
    din("xmeta", (128, 2))
    out_d = nc.dram_tensor("out", [S, T, C], U8, kind="ExternalOutput")
    osc_d = nc.dram_tensor("osc", [S, T, 1], F32, kind="ExternalOutput")

    din("wqk", (128, 2, 2 * C), BF16)
    din("qkb", (128, 4))
    din("wv", (128, 2, C), BF16)
    din("vb", (128, C))
    din("wp", (128, 2, C), BF16)
    din("pb", (128, 2))
    din("f1", (128, 2, MLP_H), BF16)
    din("f1b", (128, 8))
    din("f2", (128, 8, C), BF16)
    din("f2b", (128, 2))
    din("bmsk", (128, NBM, 512), BF16)
    din("blk", (128, 2), BF16)
    din("blkt", (2, 128), F32R)
    din("cg", (128, C))
    din("cb", (128, C))
    din("cw1", (128, 2, 2 * CAB_H))
    din("c1b", (CAB_H, 1))
    din("cw2", (CAB_H, C), BF16)
    din("c2b", (128, 2))
    din("dw", (128, 2, 9))
    din("dwb", (128, 2))

    with tile.TileContext(nc) as tc:
        _emit(tc, dram, out_d, osc_d)
    nc.compile()
    return nc


def _make_consts(p):
    g1, b1 = p["norm1_g"], p["norm1_b"]
    qkv_w2 = p["qkv_w"] * g1[None, :]
    qkv_b2 = p["qkv_b"] + p["qkv_w"] @ b1
    scale = np.float32(HD ** -0.5)
    qkv_w2[:C] *= scale
    qkv_b2[:C] *= scale

    g2, b2 = p["norm2_g"], p["norm2_b"]
    f1w = p["fc1_w"] * g2[None, :]
    f1b = p["fc1_b"] + p["fc1_w"] @ b2

    rpi = _rel_pos_index().reshape(-1)
    bias = p["rpb_table"][rpi].reshape(N, N, NH).transpose(2, 0, 1)  # (NH,N,N)
    mask = _attn_mask()                                              # (NW,N,N)
    bmsk = np.zeros((NBM, 128, 512), np.float32)

    def _hcol(h):
        return (h % 4) * 128 + (h // 4) * 64

    for h in range(NH):
        bt = bias[h].T  # [m, n]
        bmsk[0, :64, _hcol(h):_hcol(h) + 64] = bt
        bmsk[0, 64:, _hcol(h):_hcol(h) + 64] = bt
    for i, pr in enumerate(_MASKED_PAIRS):
        for w2 in range(2):
            for h in range(NH):
                bmsk[1 + i, w2 * 64:(w2 + 1) * 64, _hcol(h):_hcol(h) + 64] = (
                    bias[h].T + mask[2 * pr + w2].T)

    blk = np.zeros((128, 2), np.float32)
    blk[:64, 0] = 1.0
    blk[64:, 1] = 1.0

    # se fc1 lhsT: cols 0:16 gap branch (W1/T), cols 16:32 gmp branch (W1)
    w1 = p["cab1_w"]  # (16, 256)
    cw1 = np.zeros((2, 128, 2 * CAB_H), np.float32)
    for cbi in range(2):
        wslice = w1[:, cbi * 128:(cbi + 1) * 128].T  # [128, 16]
        cw1[cbi, :, :CAB_H] = wslice / T
        cw1[cbi, :, CAB_H:] = wslice

    dww = p["dw_w"].reshape(C, 9)
    dwst = np.zeros((128, 2, 9), np.float32)
    dwst[:, 0, :] = dww[:128]
    dwst[:, 1, :] = dww[128:]

    return dict(
        wqk=_bf(qkv_w2[:2 * C].T.reshape(2, 128, 2 * C).transpose(1, 0, 2)),
        qkb=_f32(qkv_b2[:2 * C].reshape(4, 128).T),
        wv=_bf(qkv_w2[2 * C:].T.reshape(2, 128, C).transpose(1, 0, 2)),
        vb=_f32(np.broadcast_to(qkv_b2[2 * C:], (128, C))),
        wp=_bf(p["proj_w"].T.reshape(2, 128, C).transpose(1, 0, 2)),
        pb=_f32(p["proj_b"].reshape(2, 128).T),
        f1=_bf(f1w.T.reshape(2, 128, MLP_H).transpose(1, 0, 2)),
        f1b=_f32(f1b.reshape(8, 128).T),
        f2=_bf(p["fc2_w"].T.reshape(8, 128, C).transpose(1, 0, 2)),
        f2b=_f32(p["fc2_b"].reshape(2, 128).T),
        bmsk=_bf(bmsk.transpose(1, 0, 2)),
        blk=_bf(blk),
        blkt=_f32(blk.T),
        cg=_f32(np.broadcast_to(p["cabn_g"], (128, C))),
        cb=_f32(np.broadcast_to(p["cabn_b"], (128, C))),
        cw1=_f32(cw1.transpose(1, 0, 2)),
        c1b=_f32(p["cab1_b"].reshape(CAB_H, 1)),
        cw2=_bf(p["cab2_w"].T),
        c2b=_f32(2.0 * p["cab2_b"].reshape(2, 128).T),
        dw=_f32(dwst),
        dwb=_f32(p["dw_b"].reshape(2, 128).T),
    )


def kernel(**inputs):
    p = {k: np.asarray(v, np.float32) for k, v in inputs.items()}
    x = p.pop("x")

    if "nc" not in _prog_cache:
        _prog_cache["nc"] = _build_program()
    nc = _prog_cache["nc"]

    # int8 quantize with a global scale + pre-roll on host
    ax = max(float(x.max()), -float(x.min()))
    if ax == 0.0:
        ax = 1.0
    inv = np.float32(127.0 / ax)
    if "scratch" not in _prog_cache:
        _prog_cache["scratch"] = (np.empty((B, T, C), np.float32),
                                  np.empty((B, H, W, C), np.int8))
    t, xq = _prog_cache["scratch"]
    np.multiply(x, inv, out=t)
    np.rint(t, out=t)
    # roll(-4,-4) fused into the int8 quadrant stores
    t4 = t.reshape(B, H, W, C)
    xq[:, 0:60, 0:60, :] = t4[:, 4:64, 4:64, :]
    xq[:, 0:60, 60:64, :] = t4[:, 4:64, 0:4, :]
    xq[:, 60:64, 0:60, :] = t4[:, 0:4, 4:64, :]
    xq[:, 60:64, 60:64, :] = t4[:, 0:4, 0:4, :]
    x_roll = xq.reshape(B, T, C)
    xmeta = np.broadcast_to(
        np.array([ax / 127.0, EPS * float(inv) * float(inv)], np.float32),
        (NCORES * 128, 2))
    xmeta = np.ascontiguousarray(xmeta)

    return _run_cached(nc, p, x_roll, xmeta, x)


def _get_runner(nc):
    """Build (once) a cached jitted shard_map executor for the program.

    Mirrors bass2jax.run_bass_via_pjrt's multi-core path, but keeps the
    jitted callable so repeat kernel() calls skip retracing/relowering."""
    if "runner" in _prog_cache:
        return _prog_cache["runner"]
    import jax
    from jax.sharding import Mesh, PartitionSpec
    from jax.experimental.shard_map import shard_map
    from concourse import bass2jax, mybir as _mybir
    bass2jax.install_neuronx_cc_hook()

    partition_name = nc.partition_id_tensor.name if nc.partition_id_tensor else None
    in_names, out_names, out_avals, zero_shapes = [], [], [], []
    for alloc in nc.m.functions[0].allocations:
        if not isinstance(alloc, _mybir.MemoryLocationSet):
            continue
        name = alloc.memorylocations[0].name
        if alloc.kind == "ExternalInput":
            if name != partition_name:
                in_names.append(name)
        elif alloc.kind == "ExternalOutput":
            out_names.append(name)
            out_avals.append(jax.core.ShapedArray(
                tuple(alloc.tensor_shape), _mybir.dt.np(alloc.dtype)))
            zero_shapes.append((tuple(alloc.tensor_shape),
                                _mybir.dt.np(alloc.dtype)))
    n_params = len(in_names)
    all_in = list(in_names) + list(out_names)
    if partition_name is not None:
        all_in.append(partition_name)
    donate = tuple(range(n_params, n_params + len(out_names)))

    def _body(*args):
        operands = list(args)
        if partition_name is not None:
            operands.append(bass2jax.partition_id_tensor())
        outs = bass2jax._bass_exec_p.bind(
            *operands,
            out_avals=tuple(out_avals),
            in_names=tuple(all_in),
            out_names=tuple(out_names),
            lowering_input_output_aliases=(),
            sim_require_finite=True,
            sim_require_nnan=True,
            nc=nc,
        )
        return tuple(outs)

    devices = jax.devices()[:NCORES]
    mesh = Mesh(np.asarray(devices), ("core",))
    in_specs = (PartitionSpec("core"),) * (n_params + len(out_names))
    out_specs = (PartitionSpec("core"),) * len(out_names)
    sharded = jax.jit(
        shard_map(_body, mesh=mesh, in_specs=in_specs, out_specs=out_specs,
                  check_rep=False),
        donate_argnums=donate, keep_unused=True)
    from jax.sharding import NamedSharding
    runner = dict(fn=sharded, in_names=in_names, out_names=out_names,
                  zero_shapes=zero_shapes, n_params=n_params,
                  sharding=NamedSharding(mesh, PartitionSpec("core")))
    _prog_cache["runner"] = runner
    return runner


def _consts_on_device(r, p):
    """Device-resident replicated consts, cached across calls by content."""
    import hashlib
    import jax
    h = hashlib.blake2b(digest_size=16)
    for k in sorted(p):
        h.update(k.encode())
        h.update(np.ascontiguousarray(p[k]).tobytes())
    fp = h.hexdigest()
    cached = _prog_cache.get("consts_dev")
    if cached is not None and cached[0] == fp:
        return cached[1]
    consts = _make_consts(p)
    dev = {}
    for name, arr in consts.items():
        stacked = np.ascontiguousarray(
            np.broadcast_to(arr[None], (NCORES, *arr.shape)).reshape(
                NCORES * arr.shape[0], *arr.shape[1:]))
        dev[name] = jax.device_put(stacked, r["sharding"])
    jax.block_until_ready(list(dev.values()))
    _prog_cache["consts_dev"] = (fp, dev)
    return dev


def _run_cached(nc, p, x_roll, xmeta, x):
    import jax
    import jax.numpy as jnp
    from concurrent.futures import ThreadPoolExecutor

    r = _get_runner(nc)
    consts_dev = _consts_on_device(r, p)
    percall = {"x": x_roll, "xmeta": xmeta}
    inputs = [percall[name] if name in percall else consts_dev[name]
              for name in r["in_names"]]
    # donated output buffers are created on-device (no wire traffic)
    if "zeros_fn" not in _prog_cache:
        shapes = [( (NCORES * shp[0], *shp[1:]), dt)
                  for shp, dt in r["zero_shapes"]]
        _prog_cache["zeros_fn"] = jax.jit(
            lambda: tuple(jnp.zeros(s, d) for s, d in shapes),
            out_shardings=tuple(r["sharding"] for _ in shapes))
    zeros_dev = _prog_cache["zeros_fn"]()
    out_arrs = r["fn"](*inputs, *zeros_dev)
    u_dev = out_arrs[r["out_names"].index("out")]
    a_dev = out_arrs[r["out_names"].index("osc")]
    u_shards = sorted(u_dev.addressable_shards,
                      key=lambda sh: sh.index[0].start or 0)
    a_shards = sorted(a_dev.addressable_shards,
                      key=lambda sh: sh.index[0].start or 0)
    out = np.empty((B, T, C), np.float32)
    out4 = out.reshape(B, H, W, C)
    x4 = x.reshape(B, H, W, C)

    # fetch shards concurrently (I/O-bound on the relay); reconstruct
    # serially in the main thread as each pair lands (host has 1 CPU):
    # d = (128-u)*a/126.99, un-roll, add exact f32 x
    if "fetch_pool" not in _prog_cache:
        _prog_cache["fetch_pool"] = ThreadPoolExecutor(2 * NCORES)
    ex = _prog_cache["fetch_pool"]
    futs_u = [ex.submit(lambda sh=sh: np.asarray(sh.data)) for sh in u_shards]
    futs_a = [ex.submit(lambda sh=sh: np.asarray(sh.data)) for sh in a_shards]
    if "uf_buf" not in _prog_cache:
        _prog_cache["uf_buf"] = np.empty((S, T, C), np.float32)
    uf = _prog_cache["uf_buf"]
    uf4 = uf.reshape(S, H, W, C)
    for i in range(NCORES):
        u = futs_u[i].result()                        # [S,T,C] uint8
        a = futs_a[i].result()                        # [S,T,1] f32
        np.copyto(uf, u)
        np.subtract(128.0, uf, out=uf)
        uf *= a * np.float32(1.0 / 126.99)
        # un-roll (+4,+4) fused into quadrant adds with exact f32 x
        xs, os_ = x4[i * S:(i + 1) * S], out4[i * S:(i + 1) * S]
        np.add(xs[:, 4:, 4:], uf4[:, :60, :60], out=os_[:, 4:, 4:])
        np.add(xs[:, 4:, :4], uf4[:, :60, 60:], out=os_[:, 4:, :4])
        np.add(xs[:, :4, 4:], uf4[:, 60:, :60], out=os_[:, :4, 4:])
        np.add(xs[:, :4, :4], uf4[:, 60:, 60:], out=os_[:, :4, :4])
    return out

